# revision 32
# baseline (speedup 1.0000x reference)
"""Trainium2 Bass kernel for the DAMPS spectral-fusion module.

Takes the full (unsharded) inputs, shards rows across 8 NeuronCores
(pure data parallel), and runs a fused matmul-only reformulation:

  proj + rFFT + phase-rotation  ->  one [raw, 128] matrix per modality
  (spectral packing: p0 = DC (unrotated), p1..63 = Re A_k, p64 = Nyquist
  (unrotated), p65..127 = Im A_k; cos(phi) for DC/Nyquist is folded into
  the output matrix because irfft ignores the imaginary part there)

  msc mask chain               ->  elementwise on [128, rows] tiles
                                   + tiny matmuls for the pair-sum
                                   (|A|^2) and the bin->dim mask expand
  masked irfft                 ->  one [128, 128] fp32 output matrix

Layout strategy: the host pre-transposes each core's row-shard to
[raw, rows] and splits it into two exact bf16 planes (hi = bf16(x),
lo = bf16(x - hi)) — the same total HBM bytes as fp32.  The forward
projection then runs as three bf16 matmuls per K-chunk
(Xhi@Ghi + Xhi@Glo + Xlo@Ghi; bf16 products are exact, accumulation is
fp32 in PSUM) with no on-chip transposes or PSUM staging copies at all.
The inverse transform writes the output transposed ([D, rows]); the
host transposes it back during the gather.
"""

import math

import numpy as np

N_ITEMS = 30000
D = 128
F = 65
RAW_IMG = 1024
RAW_TXT = 768
N_CORES = 8
ROWS_CORE = N_ITEMS // N_CORES          # 3750
ROWS_PAD = 3840                         # 30 * 128
KC_IMG = RAW_IMG // 128                 # 8
KC_TXT = RAW_TXT // 128                 # 6
EPS = 1e-8

# row blocks per core: 7 x 512 + 1 x 256
BLOCKS = [(i * 512, 512) for i in range(7)] + [(3584, 256)]

_CACHE = {}


def _bin_of_dim():
    """spectral dim (0..127) -> frequency bin (0..64)"""
    b = np.zeros(128, np.int64)
    b[0] = 0
    b[64] = 64
    b[1:64] = np.arange(1, 64)
    b[65:128] = np.arange(1, 64)
    return b


def _host_consts(W_img, b_img, W_txt, b_txt, avg_R, psi, lambda_weights):
    """Build the fused constant matrices (float64 internally)."""

    n = np.arange(D)
    k = np.arange(F)
    theta = 2.0 * np.pi * np.outer(k, n) / D          # [65, 128]
    phi = (avg_R.astype(np.float64) * 0.5 + psi.astype(np.float64))  # [65]
    s = 1.0 / math.sqrt(D)

    def dmat(sign):
        Dm = np.zeros((128, D))
        Dm[0, :] = s
        Dm[64, :] = s * np.cos(theta[64])
        a = theta[1:64] + sign * phi[1:64, None]
        Dm[1:64, :] = s * np.cos(a)
        Dm[65:128, :] = -s * np.sin(a)
        return Dm

    Dimg = dmat(+1.0)
    Dtxt = dmat(-1.0)

    GimgT = (W_img.astype(np.float64) @ Dimg.T).astype(np.float32)  # [1024,128]
    GtxtT = (W_txt.astype(np.float64) @ Dtxt.T).astype(np.float32)  # [768,128]
    bias_img = (Dimg @ b_img.astype(np.float64)).astype(np.float32)
    bias_txt = (Dtxt @ b_txt.astype(np.float64)).astype(np.float32)

    cphi = np.cos(phi)
    Mout = np.zeros((128, D))
    Mout[0, :] = s * cphi[0]
    Mout[64, :] = s * cphi[64] * np.cos(theta[64])
    Mout[1:64, :] = 2.0 * s * np.cos(theta[1:64])
    Mout[65:128, :] = -2.0 * s * np.sin(theta[1:64])
    Mout = Mout.astype(np.float32)

    lw = lambda_weights.astype(np.float64)
    e = np.exp(lw - lw.max())
    lam = e / e.sum()
    lam0, lam1 = float(lam[0]), float(lam[1])

    bod = _bin_of_dim()
    epair = np.zeros((128, F), np.float32)
    epair[np.arange(F), np.arange(F)] = 1.0            # cos^2 / DC / Nyq
    epair[64 + np.arange(1, 64), np.arange(1, 64)] = 1.0  # sin^2
    # e2 expand matrix maps r[bin] -> -c1*eps*r at each spectral dim; the
    # constant (c0[bin]+c1) term is applied later as a per-partition scalar:
    #   g_exp[d] = (c0[bod d]+c1) - c1*eps*r[bod d]   (== c0 + c1*msc)
    e2 = np.zeros((2, F, 128), np.float32)
    e2[0, bod, np.arange(128)] = -lam1 * EPS
    e2[1, bod, np.arange(128)] = -lam1 * EPS
    return GimgT, GtxtT, bias_img, bias_txt, Mout, lam0, lam1, epair, e2, bod


def _build_nc():
    """Build (once) the Bass/Tile program for one core's row shard."""
    from contextlib import ExitStack

    import concourse.bass as bass
    import concourse.tile as tile
    from concourse import mybir

    f32 = mybir.dt.float32
    bf16 = mybir.dt.bfloat16
    AF = mybir.ActivationFunctionType

    nc = bass.Bass("TRN2", target_bir_lowering=False, debug=False)

    # transposed bf16 hi/lo table shards, packed [2(hi/lo), raw, rows]
    xi = nc.dram_tensor("xi", [2, RAW_IMG, ROWS_PAD], bf16, kind="ExternalInput").ap()
    xt = nc.dram_tensor("xt", [2, RAW_TXT, ROWS_PAD], bf16, kind="ExternalInput").ap()
    # G hi/lo planes: [2, KC, 128, 128]
    g_img = nc.dram_tensor(
        "g_img", [2, KC_IMG, 128, 128], bf16, kind="ExternalInput"
    ).ap()
    g_txt = nc.dram_tensor(
        "g_txt", [2, KC_TXT, 128, 128], bf16, kind="ExternalInput"
    ).ap()
    mout = nc.dram_tensor("mout", [128, 128], f32, kind="ExternalInput").ap()
    epair = nc.dram_tensor("epair", [128, F], bf16, kind="ExternalInput").ap()
    e2 = nc.dram_tensor("e2", [2, F, 128], bf16, kind="ExternalInput").ap()
    biases = nc.dram_tensor("biases", [128, 4], f32, kind="ExternalInput").ap()
    # outputs stored transposed: [D, rows]
    out_img = nc.dram_tensor("out_img", [D, ROWS_PAD], f32, kind="ExternalOutput").ap()
    out_txt = nc.dram_tensor("out_txt", [D, ROWS_PAD], f32, kind="ExternalOutput").ap()

    # [128 partitions, hi/lo, K-chunk, rows] views of the transposed tables
    xiv = xi.rearrange("h (c k) r -> k h c r", k=128)
    xtv = xt.rearrange("h (c k) r -> k h c r", k=128)

    with tile.TileContext(nc) as tc, ExitStack() as ctx:
        singles = ctx.enter_context(tc.tile_pool(name="singles", bufs=1))
        xi_pool = ctx.enter_context(tc.tile_pool(name="xi", bufs=2))
        xt_pool = ctx.enter_context(tc.tile_pool(name="xt", bufs=2))
        s_pool = ctx.enter_context(tc.tile_pool(name="s", bufs=4))
        sq_pool = ctx.enter_context(tc.tile_pool(name="sq", bufs=3))
        mid_sb = ctx.enter_context(tc.tile_pool(name="midsb", bufs=2))
        mask_pool = ctx.enter_context(tc.tile_pool(name="mask", bufs=4))
        osb_pool = ctx.enter_context(tc.tile_pool(name="osb", bufs=3))

        a_ps = ctx.enter_context(tc.tile_pool(name="aps", bufs=4, space="PSUM"))
        mid_ps = ctx.enter_context(tc.tile_pool(name="midps", bufs=2, space="PSUM"))
        o_ps = ctx.enter_context(tc.tile_pool(name="ops", bufs=2, space="PSUM"))

        # ---- constants into SBUF (once) ----
        g_img_sb = singles.tile([128, 2, KC_IMG, 128], bf16)
        nc.sync.dma_start(out=g_img_sb, in_=g_img.rearrange("h c k m -> k h c m"))
        g_txt_sb = singles.tile([128, 2, KC_TXT, 128], bf16)
        nc.sync.dma_start(out=g_txt_sb, in_=g_txt.rearrange("h c k m -> k h c m"))
        mout_sb = singles.tile([128, 128], f32)
        nc.sync.dma_start(out=mout_sb, in_=mout)
        epair_sb = singles.tile([128, F], bf16)
        nc.sync.dma_start(out=epair_sb, in_=epair)
        e2i_sb = singles.tile([F, 128], bf16)
        nc.sync.dma_start(out=e2i_sb, in_=e2[0])
        e2t_sb = singles.tile([F, 128], bf16)
        nc.sync.dma_start(out=e2t_sb, in_=e2[1])
        bias_sb = singles.tile([128, 4], f32)
        nc.sync.dma_start(out=bias_sb, in_=biases)
        eps_sb = singles.tile([128, 1], f32)
        nc.vector.memset(eps_sb, EPS)

        def fwd(hv, g_sb, kc, r0, RB, tag, pool, dma_eng):
            """one packed hi/lo load + 3 exact-split bf16 matmuls per chunk"""
            x = pool.tile([128, 2, kc, RB], bf16, tag=tag)
            dma_eng.dma_start(out=x, in_=hv[:, :, :, r0 : r0 + RB])
            A = a_ps.tile([128, RB], f32, tag="A")
            for kk in range(kc):
                nc.tensor.matmul(
                    A, g_sb[:, 0, kk, :], x[:, 0, kk, :], start=(kk == 0), stop=False
                )
                nc.tensor.matmul(
                    A, g_sb[:, 1, kk, :], x[:, 0, kk, :], start=False, stop=False
                )
                nc.tensor.matmul(
                    A, g_sb[:, 0, kk, :], x[:, 1, kk, :], start=False,
                    stop=(kk == kc - 1),
                )
            return A

        def tail(A_i, A_t, r0, RB):
            # s = A + bias  (PSUM -> SBUF, on ACT: Identity with AP bias)
            s_i = s_pool.tile([128, RB], f32, tag="s", name="s_i")
            nc.scalar.activation(
                out=s_i, in_=A_i, func=AF.Identity, bias=bias_sb[:, 0:1], scale=1.0
            )
            s_t = s_pool.tile([128, RB], f32, tag="s", name="s_t")
            nc.scalar.activation(
                out=s_t, in_=A_t, func=AF.Identity, bias=bias_sb[:, 1:2], scale=1.0
            )

            # squares (bf16 is ample for the msc chain)
            sq_i = sq_pool.tile([128, RB], bf16, tag="sq", name="sq_i")
            nc.scalar.activation(out=sq_i, in_=s_i, func=AF.Square)
            sq_t = sq_pool.tile([128, RB], bf16, tag="sq", name="sq_t")
            nc.vector.tensor_mul(sq_t, s_t, s_t)

            # |A|^2 per bin via 0/1 pair-sum matrix
            a2_i = mid_ps.tile([F, RB], f32, tag="mid", name="a2_i")
            nc.tensor.matmul(a2_i, epair_sb, sq_i, start=True, stop=True)
            a2_t = mid_ps.tile([F, RB], f32, tag="mid", name="a2_t")
            nc.tensor.matmul(a2_t, epair_sb, sq_t, start=True, stop=True)

            a2t_sb = mid_sb.tile([F, RB], f32, tag="a2t", name="a2t_sb")
            nc.vector.tensor_copy(out=a2t_sb, in_=a2_t)
            p_sb = mid_sb.tile([F, RB], f32, tag="p", name="p_sb")
            nc.vector.tensor_mul(p_sb, a2_i, a2t_sb)

            # r = 1/(p+eps) as exp(-ln(p+eps)) on ACT
            u_sb = mid_sb.tile([F, RB], f32, tag="u", name="u_sb")
            nc.scalar.activation(
                out=u_sb, in_=p_sb, func=AF.Ln, bias=eps_sb[:F], scale=1.0
            )
            r_bf = mid_sb.tile([F, RB], bf16, tag="rbf", name="r_bf")
            nc.scalar.activation(out=r_bf, in_=u_sb, func=AF.Exp, bias=0.0, scale=-1.0)

            # g expand (variable part): [65] -> [128] dims
            gexp_i = mid_ps.tile([128, RB], f32, tag="mid", name="gexp_i")
            nc.tensor.matmul(gexp_i, e2i_sb, r_bf, start=True, stop=True)
            gexp_t = mid_ps.tile([128, RB], f32, tag="mid", name="gexp_t")
            nc.tensor.matmul(gexp_t, e2t_sb, r_bf, start=True, stop=True)

            # masked spectra: (gexp + cc_col) * s
            mask_i = mask_pool.tile([128, RB], f32, tag="mask", name="mask_i")
            nc.vector.scalar_tensor_tensor(
                out=mask_i, in0=gexp_i, scalar=bias_sb[:, 2:3], in1=s_i,
                op0=mybir.AluOpType.add, op1=mybir.AluOpType.mult,
            )
            mask_t = mask_pool.tile([128, RB], f32, tag="mask", name="mask_t")
            nc.vector.scalar_tensor_tensor(
                out=mask_t, in0=gexp_t, scalar=bias_sb[:, 3:4], in1=s_t,
                op0=mybir.AluOpType.add, op1=mybir.AluOpType.mult,
            )

            # inverse transform (fp32 exact), output transposed [D, rows]
            o_i = o_ps.tile([128, RB], f32, tag="o", name="o_i")
            nc.tensor.matmul(o_i, mout_sb, mask_i, start=True, stop=True)
            o_t = o_ps.tile([128, RB], f32, tag="o", name="o_t")
            nc.tensor.matmul(o_t, mout_sb, mask_t, start=True, stop=True)

            osb_i = osb_pool.tile([128, RB], f32, tag="osb", name="osb_i")
            nc.scalar.copy(out=osb_i, in_=o_i)
            osb_t = osb_pool.tile([128, RB], f32, tag="osb", name="osb_t")
            nc.vector.tensor_copy(out=osb_t, in_=o_t)

            nc.sync.dma_start(out=out_img[:, r0 : r0 + RB], in_=osb_i)
            nc.gpsimd.dma_start(out=out_txt[:, r0 : r0 + RB], in_=osb_t)

        # software pipeline: fwd(b+1) issues before the spectral tail of b,
        # so PE does not stall on the ACT/DVE elementwise chain latency.
        pending = None
        for (r0, RB) in BLOCKS:
            A_i = fwd(xiv, g_img_sb, KC_IMG, r0, RB, "xi", xi_pool, nc.sync)
            A_t = fwd(xtv, g_txt_sb, KC_TXT, r0, RB, "xt", xt_pool, nc.gpsimd)
            if pending is not None:
                tail(*pending)
            pending = (A_i, A_t, r0, RB)
        tail(*pending)

    _legalize_waits(nc)
    return nc


def _legalize_waits(nc):
    """This toolchain's walrus accepts at most ONE sync-wait command per
    engine instruction. Hoist excess waits onto same-engine EventSemaphore
    instructions inserted immediately before the offending instruction
    (engines execute their stream in order, so the carrier's wait gates
    the next instruction too)."""
    import bass_rust

    k = 0
    for f in nc.m.functions:
        for bb in f.blocks:
            new = []
            for ins in bb.instructions:
                si = getattr(ins, "sync_info", None)
                waits = list(si.on_wait) if si is not None and si.on_wait else []
                if len(waits) > 1:
                    for w in waits[:-1]:
                        nop = bass_rust.InstEventSemaphore(name=f"I-legalw-{k}")
                        k += 1
                        nop.engine = ins.engine
                        nop.sync_info = bass_rust.SyncInfo(on_wait=[w], on_update=[])
                        new.append(nop)
                    ins.sync_info = bass_rust.SyncInfo(
                        on_wait=[waits[-1]], on_update=list(si.on_update)
                    )
                new.append(ins)
            bb.instructions = new


LAST_RESULTS = None


def kernel(
    image_embeds,
    text_embeds,
    image_table,
    text_table,
    W_img,
    b_img,
    W_txt,
    b_txt,
    avrf_img,
    avrf_txt,
    avg_R,
    psi,
    lambda_weights,
):
    global LAST_RESULTS
    import ml_dtypes
    from concourse.bass_utils import run_bass_kernel_spmd

    bf = ml_dtypes.bfloat16
    image_table = np.asarray(image_table, np.float32)
    text_table = np.asarray(text_table, np.float32)
    W_img = np.asarray(W_img, np.float32)
    b_img = np.asarray(b_img, np.float32)
    W_txt = np.asarray(W_txt, np.float32)
    b_txt = np.asarray(b_txt, np.float32)
    avrf_img = np.asarray(avrf_img, np.float32)
    avrf_txt = np.asarray(avrf_txt, np.float32)
    avg_R = np.asarray(avg_R, np.float32)
    psi = np.asarray(psi, np.float32)
    lambda_weights = np.asarray(lambda_weights, np.float32)

    (GimgT, GtxtT, bias_img, bias_txt, Mout, lam0, lam1, epair, e2, bod) = _host_consts(
        W_img, b_img, W_txt, b_txt, avg_R, psi, lambda_weights
    )
    # per-partition constant term of the mask: cc[d] = c0[bod d] + c1
    cc_img = (lam0 * avrf_img[bod] + lam1).astype(np.float32)
    cc_txt = (lam0 * avrf_txt[bod] + lam1).astype(np.float32)
    biases = np.stack([bias_img, bias_txt, cc_img, cc_txt], axis=1).astype(
        np.float32
    )  # [128, 4]

    def hilo_g(g, kc):
        hi = g.astype(bf).astype(np.float32)
        lo = (g - hi).astype(np.float32)
        return np.ascontiguousarray(
            np.stack([hi.reshape(kc, 128, 128), lo.reshape(kc, 128, 128)]).astype(bf)
        )

    g_img_c = hilo_g(GimgT, KC_IMG)
    g_txt_c = hilo_g(GtxtT, KC_TXT)

    # full-table transposed bf16 hi/lo planes (exact split: x == hi + lo
    # to ~2^-18 relative)
    def hilo_T(x):
        hi = x.astype(bf)
        lo = (x - hi.astype(np.float32)).astype(bf)
        return np.ascontiguousarray(hi.T), np.ascontiguousarray(lo.T)

    xih_full, xil_full = hilo_T(image_table)   # [1024, 30000]
    xth_full, xtl_full = hilo_T(text_table)    # [768, 30000]
    xi_full = np.stack([xih_full, xil_full])   # [2, 1024, 30000]
    xt_full = np.stack([xth_full, xtl_full])   # [2, 768, 30000]

    if "nc" not in _CACHE:
        _CACHE["nc"] = _build_nc()
    nc = _CACHE["nc"]

    consts = dict(
        g_img=g_img_c,
        g_txt=g_txt_c,
        mout=np.ascontiguousarray(Mout, dtype=np.float32),
        epair=np.ascontiguousarray(epair, dtype=bf),
        e2=np.ascontiguousarray(e2, dtype=bf),
        biases=biases,
    )

    def shard(full, raw):
        out = []
        for c in range(N_CORES):
            lo_i = c * ROWS_CORE
            arr = np.zeros((2, raw, ROWS_PAD), bf)
            arr[:, :, :ROWS_CORE] = full[:, :, lo_i : lo_i + ROWS_CORE]
            out.append(arr)
        return out

    xi_s = shard(xi_full, RAW_IMG)
    xt_s = shard(xt_full, RAW_TXT)

    in_maps = [
        dict(xi=xi_s[c], xt=xt_s[c], **consts) for c in range(N_CORES)
    ]

    res = run_bass_kernel_spmd(nc, in_maps, core_ids=list(range(N_CORES)))
    LAST_RESULTS = res

    img = np.concatenate(
        [res.results[c]["out_img"][:, :ROWS_CORE].T for c in range(N_CORES)], axis=0
    ).astype(np.float32)
    txt = np.concatenate(
        [res.results[c]["out_txt"][:, :ROWS_CORE].T for c in range(N_CORES)], axis=0
    ).astype(np.float32)
    return img, txt
